# revision 53
# baseline (speedup 1.0000x reference)
"""Trainium2 Bass kernel for CapsuleLayer dynamic routing (v2).

Problem: x [64, 2048, 16], W [1, 2048, 32, 32, 16] ->
  u_hat = einsum('bik,ijdk->bijd', x, W[0])           [B, N_in, N_out, D_out]
  3 rounds of routing (softmax over j, weighted sum over i, squash),
  returns v [64, 32, 32].

Sharding: N_in (2048) split over 8 cores, 256 input capsules each. Each core
computes partial s/Z sums over its i-range; an AllReduce combines the
partials; every core then computes the identical squash updates.

v2 vs v1: routing contractions moved off the DVE onto the PE.
 - weighted sum: prod_s = e * U (DVE 2x; e stored as duplicated pairs so the
   d-broadcast AP keeps a stride-1 innermost dim), then PSUM-accumulated
   matmuls with the block-diagonal ones32 stationary sum over (blk,h) AND the
   4 partition g-groups in one chain -> s' lands in PSUM.  No DVE reduce.
 - agreement: prod_a = U * V_rep (DVE 2x), then 32 PSUM-accumulated matmuls
   (one per d, identity stationary, moving AP strided by D_OUT) produce the
   logits in PSUM; exp runs on ACT straight out of PSUM into e2.
 - softmax normalization folded into the squash scalars (s = s'/Z); the Z
   partials ride the same AllReduce payload as s'.
 - running logits replaced by V_acc = sum_t v_t (agreement is linear in v).
"""
import sys

sys.path.insert(0, '/opt/trn_rl_repo')

import numpy as np

import concourse.bass as bass
import concourse.mybir as mybir
from concourse import bass_utils, tile

# ---------------------------------------------------------------- constants
N_CORES = 8
B = 64
N_IN = 2048
D_IN = 16
N_OUT = 32
D_OUT = 32
ROUTINGS = 3
EPS = 1e-9

I_LOC = N_IN // N_CORES          # 256
NBLK = I_LOC // 8                # 32 blocks of 8 input capsules
BC = 32                          # batch chunk
NCHUNK = B // BC                 # 2
JD = N_OUT * D_OUT               # 1024
AGG_GRP = 4                      # blocks per agreement group
NGRP = NBLK // AGG_GRP           # 8
Z0 = float(N_OUT)                # t=0 softmax denominator (uniform over j)

f32 = mybir.dt.float32
bf16 = mybir.dt.bfloat16

DEBUG = 0  # 1: add per-round s'/Z taps as extra outputs

_MAX_WAITS = 1
_carrier = [0]


def _patch_tile():
    """Work around this walrus build rejecting >1 sync wait per instruction."""
    import concourse.mybir as _mybir
    from concourse import tile as _tile
    from concourse.tile import TileContext as _TC

    def _drain_and_barrier(self, tick_clock, wait_clock):
        ScopedClock = _tile.ScopedClock
        probe = self.nc.sync.nop(nofuse=True)
        wait_clock.add_sem_waits(
            probe.ins, ScopedClock({None: tick_clock.global_clock})
        )
        si = probe.ins.sync_info
        waits = list(si.on_wait)
        probe.ins.sync_info = _mybir.SyncInfo(
            on_wait=waits[:1], on_update=list(si.on_update)
        )
        for w in waits[1:]:
            carrier = self.nc.sync.nop(nofuse=True)
            carrier.ins.sync_info = _mybir.SyncInfo(on_wait=[w], on_update=[])
        self.nc.sync.drain()
        self.nc.all_engine_barrier()
        assert self.sems is not None
        popped = self.nc._tile_sem_poison_stack.pop()
        assert popped is self._sem_poison
        self.nc.clear_and_free_semaphores(list(self.sems.allocated().values()))
        self.nc.all_engine_barrier()

    _TC._drain_and_barrier = _drain_and_barrier

    try:
        from concourse import tile_utils
        tile_utils.max_sbuf_usage = 208 * 1024
    except Exception:
        pass


def _fix_sync_waits(nc, max_waits=_MAX_WAITS):
    n_fixed = 0
    for func in nc.m.functions:
        for bb in func.blocks:
            insts = list(bb.instructions)
            new_list = []
            changed = False
            for inst in insts:
                si = getattr(inst, "sync_info", None)
                waits = list(si.on_wait) if si is not None else []
                if len(waits) > max_waits:
                    keep = waits[: max_waits - 1] if max_waits > 1 else []
                    hoist = waits[len(keep):-1]
                    tail = [waits[-1]]
                    for w in hoist:
                        _carrier[0] += 1
                        nop = mybir.InstNoOp(
                            name=f"syncfix-{_carrier[0]}", engine=inst.engine
                        )
                        nop.sync_info = mybir.SyncInfo(on_wait=[w], on_update=[])
                        new_list.append(nop)
                    inst.sync_info = mybir.SyncInfo(
                        on_wait=keep + tail, on_update=list(si.on_update)
                    )
                    changed = True
                    n_fixed += 1
                new_list.append(inst)
            if changed:
                bb.instructions = new_list
    return n_fixed


# ---------------------------------------------------------------- program
def _build_program():
    _patch_tile()
    nc = bass.Bass(trn_type="TRN2", num_devices=N_CORES)

    xz_in = nc.dram_tensor("xz", [128, NBLK * 2 * NCHUNK * 128], bf16,
                           kind="ExternalInput")
    xd_in = nc.dram_tensor("xd", [128, NBLK * B], bf16, kind="ExternalInput")
    wt_in = nc.dram_tensor("wt", [128, NBLK * JD], bf16, kind="ExternalInput")
    ones_in = nc.dram_tensor("ones32", [128, BC], bf16, kind="ExternalInput")
    id_in = nc.dram_tensor("ident", [128, 128], bf16, kind="ExternalInput")
    v_out = nc.dram_tensor("v", [B, JD], f32, kind="ExternalOutput")
    dbg_sq = None
    dbg_e2 = None
    if DEBUG:
        dbg_sq = nc.dram_tensor(
            "dbg_sq", [NCHUNK * ROUTINGS * BC, JD], bf16,
            kind="ExternalOutput")
        dbg_e2 = nc.dram_tensor(
            "dbg_e2", [128, NBLK * 2 * N_OUT * 2], bf16,
            kind="ExternalOutput")

    AluOp = mybir.AluOpType
    Act = mybir.ActivationFunctionType
    Axis = mybir.AxisListType
    rg = [list(range(N_CORES))]

    from contextlib import ExitStack
    with tile.TileContext(nc, num_cores=N_CORES) as tc, ExitStack() as es:
        cpool = es.enter_context(tc.tile_pool(name="const", bufs=1))
        wpool = es.enter_context(tc.tile_pool(name="wstream", bufs=2))
        upool = es.enter_context(tc.tile_pool(name="ubuf", bufs=1))
        papool = es.enter_context(tc.tile_pool(name="pagrp", bufs=2))
        pspool = es.enter_context(tc.tile_pool(name="psblk", bufs=2))
        e2pool = es.enter_context(tc.tile_pool(name="e2", bufs=1))
        zpool = es.enter_context(tc.tile_pool(name="zp", bufs=1))
        xzpool = es.enter_context(tc.tile_pool(name="xzq", bufs=1))
        sqpool = es.enter_context(tc.tile_pool(name="sq", bufs=1))
        vfpool = es.enter_context(tc.tile_pool(name="vf", bufs=1))
        vpool = es.enter_context(tc.tile_pool(name="vacc", bufs=2))
        ps1pool = es.enter_context(tc.tile_pool(name="psum1", bufs=2, space="PSUM"))
        aggpool = es.enter_context(tc.tile_pool(name="psumagg", bufs=2, space="PSUM"))
        redpool = es.enter_context(tc.tile_pool(name="psumred", bufs=1, space="PSUM"))
        dpool = es.enter_context(tc.tile_pool(name="dram", bufs=1, space="DRAM"))

        epsc = cpool.tile([128, 1], f32, tag="epsc")
        nc.vector.memset(epsc[:], EPS)
        xd = cpool.tile([128, NBLK * B], bf16)
        ones32 = cpool.tile([128, BC], bf16)
        ident = cpool.tile([128, 128], bf16)
        nc.sync.dma_start(xd[:], xd_in[:])
        nc.sync.dma_start(ones32[:], ones_in[:])
        nc.sync.dma_start(ident[:], id_in[:])

        ar0_in = dpool.tile([B, JD], f32, tag="ar0i")
        ar0_out = dpool.tile([B, JD], f32, tag="ar0o")

        for q in range(NCHUNK):
            # ================= phase 1: u_hat for this chunk (+ s0 at q=0)
            U = upool.tile([128, NBLK * 2 * JD], bf16, tag="U")
            xz = xzpool.tile([128, NBLK * 2 * 128], bf16, tag="xzq")
            nc.sync.dma_start(
                xz[:], xz_in[:, q * NBLK * 2 * 128:(q + 1) * NBLK * 2 * 128])
            if q == 0:
                ps0 = redpool.tile([B, JD], f32, tag="red")
            for blk in range(NBLK):
                w = wpool.tile([128, JD], bf16, tag="w")
                nc.sync.dma_start(w[:], wt_in[:, blk * JD:(blk + 1) * JD])
                if q == 0:
                    # s0' = sum_i u_hat (full batch, K=128-packed dense);
                    # high priority so the ar0 AllReduce fires early and
                    # hides under the rest of phase 1
                    with tc.high_priority():
                        for half in range(2):
                            nc.tensor.matmul(
                                ps0[:, half * 512:(half + 1) * 512],
                                xd[:, blk * B:(blk + 1) * B],
                                w[:, half * 512:(half + 1) * 512],
                                start=(blk == 0),
                                stop=(blk == NBLK - 1),
                            )
                for h in range(2):
                    ps = ps1pool.tile([128, JD], f32, tag="ps1")
                    # one K=128 MM per half: block-diagonal zero-padded x
                    # stationary puts all 4 g-groups in one pass
                    col = (blk * 2 + h) * 128
                    for half in range(2):
                        nc.tensor.matmul(
                            ps[:, half * 512:(half + 1) * 512],
                            xz[:, col:col + 128],
                            w[:, half * 512:(half + 1) * 512],
                            start=True, stop=True,
                        )
                    # U stored per (blk,h) in (dh, j, dl) order: d = 2*dh+dl.
                    # (j, dl) runs are contiguous 64-elem blocks, so every
                    # consumer AP fits in 3 free dims.
                    dst = U[:, (blk * 2 + h) * JD:(blk * 2 + h + 1) * JD] \
                        .rearrange("p (dh j dl) -> p dh j dl",
                                   dh=D_OUT // 2, j=N_OUT, dl=2)
                    src = ps[:].rearrange("p (j dh dl) -> p dh j dl",
                                          j=N_OUT, dh=D_OUT // 2, dl=2)
                    # PSUM -> SBUF bf16 copies: q=0 splits ACT/DVE (both
                    # otherwise idle); q=1 overlaps q=0 routing, ACT only.
                    if q == 0 and h == 0:
                        nc.vector.tensor_copy(dst, src)
                    else:
                        nc.scalar.copy(dst, src)
            if q == 0:
                # evacuate s0' permuted to (dh, j, dl) order, AllReduce (f32:
                # the CC's bf16 path is slow at this payload size)
                s0_sb = sqpool.tile([B, JD], f32, tag="sqf")
                nc.scalar.copy(
                    s0_sb[:].rearrange("p (dh j dl) -> p dh j dl",
                                       dh=D_OUT // 2, j=N_OUT, dl=2),
                    ps0[:].rearrange("p (j dh dl) -> p dh j dl",
                                     j=N_OUT, dh=D_OUT // 2, dl=2),
                )
                nc.sync.dma_start(ar0_in[:], s0_sb[:])
                nc.gpsimd.collective_compute(
                    "AllReduce", AluOp.add, replica_groups=rg,
                    ins=[ar0_in.opt()], outs=[ar0_out.opt()],
                )
            # ================= routing for this chunk
            v_acc = None
            v_rep = None
            for t in range(ROUTINGS):
                if t > 0:
                    # ---- agreement logits a[(g,b'),(blk,h,j)] =
                    #      sum_d U * V_acc  (mult on DVE, d-sum on PE)
                    e2 = e2pool.tile([128, NBLK * 2 * N_OUT * 2], bf16,
                                     tag="e2")
                    zi = zpool.tile([128, NBLK * 2], f32, tag="zi")
                    zr = zpool.tile([128, NBLK * 2], f32, tag="zr")
                    zr2 = zpool.tile([128, NBLK * 2 * 2], bf16, tag="zr2")
                    for grp in range(NGRP):
                        nbh = AGG_GRP * 2
                        off = grp * AGG_GRP * 2 * JD
                        DH = D_OUT // 2
                        DHH = DH // 2
                        NW = nbh * 2 * N_OUT  # moving cols per d_hi slice
                        # prod_a laid out (d_hi, bh, j, d_lo=2): the mult
                        # keeps a stride-1 innermost pair (2x mode) AND each
                        # fixed-d_hi slice is a contiguous 512-col MM moving.
                        # Split by d_hi halves to halve the pa footprint.
                        apg = aggpool.tile([128, 512], f32, tag="apg")
                        for dhh in range(2):
                            pa = papool.tile([128, AGG_GRP * JD], bf16,
                                             tag="pa")
                            Ug = U[:, off:off + nbh * JD].rearrange(
                                "p (bh dh jd) -> p dh bh jd",
                                bh=nbh, dh=DH, jd=2 * N_OUT,
                            )[:, dhh * DHH:(dhh + 1) * DHH]
                            v4 = (
                                v_rep[:]
                                .rearrange("p (dh jd) -> p dh jd",
                                           dh=DH, jd=2 * N_OUT)
                                [:, dhh * DHH:(dhh + 1) * DHH]
                                .unsqueeze(2)
                                .to_broadcast((128, DHH, nbh, 2 * N_OUT))
                            )
                            pa4 = pa[:].rearrange(
                                "p (dh bh jd) -> p dh bh jd",
                                dh=DHH, bh=nbh, jd=2 * N_OUT,
                            )
                            nc.vector.tensor_tensor(pa4, Ug, v4, AluOp.mult)
                            # PE d_hi-accumulation: contiguous N=512 MMs
                            for dh in range(DHH):
                                nc.tensor.matmul(
                                    apg[:, :],
                                    ident[:],
                                    pa[:, dh * NW:(dh + 1) * NW],
                                    start=(dhh == 0 and dh == 0),
                                    stop=(dhh == 1 and dh == DHH - 1),
                                )
                        # pair-add d_lo, then exp into duplicated pairs
                        a2 = zpool.tile([128, AGG_GRP * 2 * N_OUT], f32,
                                        tag="a2")
                        apv = apg[:].rearrange(
                            "p (bh j dl) -> p bh j dl",
                            bh=nbh, j=N_OUT, dl=2,
                        )
                        a23 = a2[:].rearrange(
                            "p (bh j) -> p bh j", bh=nbh, j=N_OUT,
                        )
                        nc.vector.reduce_sum(a23, apv, axis=Axis.X)
                        eoff = grp * nbh * N_OUT * 2
                        e2g = e2[:, eoff:eoff + nbh * N_OUT * 2].rearrange(
                            "p (x r) -> p x r", x=nbh * N_OUT, r=2,
                        )
                        for r in range(2):
                            nc.scalar.activation(
                                e2g[:, :, r], a2[:], Act.Exp
                            )
                        # softmax normalization over j (per b,i — local)
                        goff = grp * nbh
                        e2g3 = e2[:, eoff:eoff + nbh * N_OUT * 2].rearrange(
                            "p (bh j r) -> p bh j r", bh=nbh, j=N_OUT, r=2,
                        )
                        nc.vector.reduce_sum(
                            zi[:, goff:goff + nbh],
                            e2g3[:, :, :, 0], axis=Axis.X,
                        )
                        nc.vector.reciprocal(
                            zr[:, goff:goff + nbh], zi[:, goff:goff + nbh]
                        )
                        zr2g = zr2[:, goff * 2:(goff + nbh) * 2].rearrange(
                            "p (bh r) -> p bh r", bh=nbh, r=2,
                        )
                        nc.vector.tensor_copy(
                            zr2g,
                            zr[:, goff:goff + nbh].unsqueeze(2)
                            .to_broadcast((128, nbh, 2)),
                        )
                        # c = e / Z_i, in place over e2
                        nc.vector.tensor_tensor(
                            e2g3,
                            e2g3,
                            zr2g.unsqueeze(2).to_broadcast(
                                (128, nbh, N_OUT, 2)),
                            AluOp.mult,
                        )

                    # ---- weighted sum s'[b',(j,d)] on PE (ones32 stationary
                    #      folds g; PSUM accumulation folds blk,h)
                    sps = redpool.tile([B, JD], f32, tag="red")
                    for blk in range(NBLK):
                        for h in range(2):
                            psb = pspool.tile([128, JD], bf16, tag="psb")
                            Ub = U[:, (blk * 2 + h) * JD:
                                   (blk * 2 + h + 1) * JD].rearrange(
                                "p (dh j dl) -> p dh j dl",
                                dh=D_OUT // 2, j=N_OUT, dl=2,
                            )
                            eb = (
                                e2[:, (blk * 2 + h) * N_OUT * 2:
                                   (blk * 2 + h + 1) * N_OUT * 2]
                                .rearrange("p (j r) -> p j r", j=N_OUT, r=2)
                                .unsqueeze(1)
                                .to_broadcast((128, D_OUT // 2, N_OUT, 2))
                            )
                            # psb in permuted (dh, j, dl) column order: the
                            # dst traversal is contiguous, keeping 2x mode.
                            # s'/v inherit this order; only the final v
                            # output converts back.
                            pb = psb[:].rearrange(
                                "p (dh j dl) -> p dh j dl",
                                dh=D_OUT // 2, j=N_OUT, dl=2,
                            )
                            nc.vector.tensor_tensor(pb, Ub, eb, AluOp.mult)
                            for half in range(2):
                                nc.tensor.matmul(
                                    sps[0:BC, half * 512:(half + 1) * 512],
                                    ones32[:],
                                    psb[:, half * 512:(half + 1) * 512],
                                    start=(blk == 0 and h == 0),
                                    stop=(blk == NBLK - 1 and h == 1),
                                )
                    # ---- evacuate s' (bf16), AllReduce
                    s_sb = sqpool.tile([B, JD], bf16, tag="sq")
                    nc.scalar.copy(s_sb[0:BC, :], sps[0:BC, :])
                    ar_in = dpool.tile([BC, JD], bf16, tag=f"ari{q}{t}")
                    ar_out = dpool.tile([BC, JD], bf16,
                                        tag=f"aro{q}{t}")
                    nc.sync.dma_start(ar_in[:], s_sb[0:BC, :])
                    nc.gpsimd.collective_compute(
                        "AllReduce", AluOp.add, replica_groups=rg,
                        ins=[ar_in.opt()], outs=[ar_out.opt()],
                    )

                # ---- fetch s' to SBUF (t=0 payload is f32)
                if t == 0:
                    sq = sqpool.tile([B, JD], f32, tag="sqf")
                    nc.sync.dma_start(
                        sq[0:BC, :], ar0_out[q * BC:(q + 1) * BC, :]
                    )
                else:
                    sq = sqpool.tile([B, JD], bf16, tag="sq")
                    nc.sync.dma_start(sq[0:BC, :], ar_out[:])

                if DEBUG and t > 0:
                    row = (q * ROUTINGS + t) * BC
                    nc.sync.dma_start(
                        dbg_sq[row:row + BC, :], sq[0:BC, :]
                    )
                    if q == 0 and t == 1:
                        nc.sync.dma_start(dbg_e2[:, :], e2[:])

                # ---- squash on [BC, *]:  s = s'/Z, v = f(|s|^2) * s
                # sq holds s' in permuted (dh, j, dl) column order
                prod_sq = pspool.tile([BC, JD], bf16, tag="psb")
                nc.scalar.square(prod_sq[:], sq[0:BC, :])
                s2p = sqpool.tile([BC, N_OUT], f32, tag="s2p")
                nc.vector.reduce_sum(
                    s2p[:],
                    prod_sq[:].rearrange("p (dh j dl) -> p j dh dl",
                                         dh=D_OUT // 2, j=N_OUT, dl=2),
                    axis=Axis.XY,
                )
                # t=0: s = s'/Z0 (uniform c) -> s2 = s2'/Z0^2, g = f/Z0
                if t == 0:
                    s2 = sqpool.tile([BC, N_OUT], f32, tag="s2")
                    nc.vector.tensor_scalar_mul(s2[:], s2p[:],
                                                1.0 / (Z0 * Z0))
                else:
                    s2 = s2p
                r1 = sqpool.tile([BC, N_OUT], f32, tag="r1")
                nc.scalar.activation(r1[:], s2[:], Act.Sqrt, bias=epsc[0:BC])
                r2 = sqpool.tile([BC, N_OUT], f32, tag="r2")
                nc.vector.tensor_scalar_add(r2[:], s2[:], 1.0 + EPS)
                r3 = sqpool.tile([BC, N_OUT], f32, tag="r3")
                nc.vector.tensor_mul(r3[:], r1[:], r2[:])
                fr = sqpool.tile([BC, N_OUT], f32, tag="fr")
                nc.vector.reciprocal(fr[:], r3[:])
                gsc = sqpool.tile([BC, N_OUT], f32, tag="gsc")
                nc.vector.tensor_mul(gsc[:], fr[:], s2[:])
                if t == 0:
                    nc.vector.tensor_scalar_mul(gsc[:], gsc[:], 1.0 / Z0)


                if t < ROUTINGS - 1:
                    # v inherits the permuted (dh, j, dl) order, matching
                    # the agreement mult's merged-jd AP
                    v32 = pspool.tile([BC, JD], bf16, tag="psb")
                    v3 = v32[:].rearrange("p (dh j dl) -> p dh j dl",
                                          dh=D_OUT // 2, j=N_OUT, dl=2)
                    sp3v = sq[0:BC, :].rearrange(
                        "p (dh j dl) -> p dh j dl",
                        dh=D_OUT // 2, j=N_OUT, dl=2)
                    g3v = (gsc[:].unsqueeze(1).unsqueeze(3)
                           .to_broadcast((BC, D_OUT // 2, N_OUT, 2)))
                    nc.vector.tensor_tensor(v3, sp3v, g3v, AluOp.mult)
                    if t == 0:
                        v_acc = vpool.tile([BC, JD], bf16, tag="va",
                                           bufs=1)
                        nc.vector.tensor_copy(v_acc[:], v32[:])
                    else:
                        nc.vector.tensor_add(v_acc[:], v_acc[:], v32[:])
                    # replicate V_acc across the 4 g partition groups
                    v_rep = vpool.tile([128, JD], bf16, tag="vrep", bufs=1)
                    for g in range(4):
                        nc.sync.dma_start(
                            v_rep[32 * g:32 * g + 32, :], v_acc[:]
                        )
                else:
                    # final v in canonical (j, d) order for the output
                    v_fin = sqpool.tile([BC, JD], f32, tag="sqf")
                    vf3 = v_fin[:].rearrange("p (j dh dl) -> p dh j dl",
                                             j=N_OUT, dh=D_OUT // 2, dl=2)
                    sp3f = sq[0:BC, :].rearrange(
                        "p (dh j dl) -> p dh j dl",
                        dh=D_OUT // 2, j=N_OUT, dl=2)
                    g3f = (gsc[:].unsqueeze(1).unsqueeze(3)
                           .to_broadcast((BC, D_OUT // 2, N_OUT, 2)))
                    nc.vector.tensor_tensor(vf3, sp3f, g3f, AluOp.mult)
                    nc.sync.dma_start(
                        v_out[q * BC:(q + 1) * BC, :], v_fin[:]
                    )

    _fix_sync_waits(nc)
    return nc


# ---------------------------------------------------------------- host prep
def _prep_inputs(x, W):
    """Build per-core input maps. x [B, N_in, D_in] f32, W [1,...] f32."""
    import jax.numpy as jnp

    def tobf(a):
        return np.asarray(jnp.asarray(a).astype(jnp.bfloat16))

    in_maps = []
    ones32 = np.zeros((128, BC), np.float32)
    for p in range(128):
        ones32[p, p % 32] = 1.0
    ident = np.eye(128, dtype=np.float32)
    for c in range(N_CORES):
        xi = x[:, c * I_LOC:(c + 1) * I_LOC, :]          # [B, 256, 16]
        wi = W[0, c * I_LOC:(c + 1) * I_LOC]             # [256, 32, 32, 16]
        # (block, g, h) indexing of local capsule i = blk*8 + g*2 + h
        x6 = xi.reshape(B, NBLK, 4, 2, D_IN)             # b, blk, g, h, k
        w6 = wi.reshape(NBLK, 4, 2, N_OUT, D_OUT, D_IN)  # blk,g,h,j,d,k

        # wt[32g+16h+k, blk*1024 + j*32 + d]
        wt = np.transpose(w6, (1, 2, 5, 0, 3, 4)).reshape(128, NBLK * JD)

        # xz[32g'+16h'+k', ((blk*2+h)*2+q)*128 + 32g+b'] =
        #   x[q*32+b', blk*8+g*2+h, k']  iff (g',h')==(g,h) else 0
        xz = np.zeros((4, 2, D_IN, NBLK, 2, NCHUNK, 4, BC), np.float32)
        xq = x6.reshape(NCHUNK, BC, NBLK, 4, 2, D_IN)    # q,b',blk,g,h,k
        for g in range(4):
            for h in range(2):
                sub = xq[:, :, :, g, h, :]               # q, b', blk, k
                xz[g, h, :, :, h, :, g, :] = np.transpose(
                    sub, (3, 2, 0, 1))                   # k, blk, q, b'
        xz = xz.reshape(128, NBLK, 2, NCHUNK, 128)
        xz = np.transpose(xz, (0, 3, 1, 2, 4)).reshape(
            128, NCHUNK * NBLK * 2 * 128)
        xt = np.transpose(x6, (2, 3, 4, 1, 0))           # g, h, k, blk, b

        # xd[32g+16h+k, blk*64 + b] = x  (dense, unscaled; t=0 uses Z0=N_in)
        xd = xt.reshape(128, NBLK * B)

        in_maps.append({
            "xz": tobf(np.ascontiguousarray(xz)),
            "xd": tobf(np.ascontiguousarray(xd)),
            "wt": tobf(np.ascontiguousarray(wt)),
            "ones32": tobf(ones32),
            "ident": tobf(ident),
        })
    return in_maps


_cached = {}


def _get_program():
    if "nc" not in _cached:
        _cached["nc"] = _build_program()
    return _cached["nc"]


def kernel(x, W):
    x = np.asarray(x, dtype=np.float32)
    W = np.asarray(W, dtype=np.float32)
    nc = _get_program()
    in_maps = _prep_inputs(x, W)
    res = bass_utils.run_bass_kernel_spmd(
        nc, in_maps, core_ids=list(range(N_CORES))
    )
    v = res.results[0]["v"].reshape(B, N_OUT, D_OUT)
    return v.astype(np.float32)
